# revision 5
# baseline (speedup 1.0000x reference)
"""Trainium2 Bass kernel for nn_AttentionModeEncoder (B=4, S=2048, HID=1024, 16 heads x 64).

Sharding: 8 cores = 4 batches x 2 head-groups (8 heads / 512 features per core).
Each core computes, for its batch b and head-group g:
  - K^T, V (ones-augmented) and Q^T projections for its 512 features, all 2048 tokens
  - masked softmax attention in transposed layout:
      S^T[k,q] = K^T.T @ Q^T (d=64 contraction), P = exp(0.125*S + maskbias) via
      ScalarE with the additive mask as the per-partition bias, AV out^T[d,q] with a
      ones row in V giving softmax denominators for free, PE outer-product broadcast
      + DVE reciprocal/multiply for the normalize
  - partial out-projection y^T[o,t] = Wo[:, cslice] @ attn^T  (+bo on g==0 cores)
Host sums the two partials per batch (the cross-head-group reduction) and transposes.
"""

import os
import sys
import numpy as np
from contextlib import ExitStack

for _p in ("/opt/trn_rl_repo", "/root/.axon_site/_ro/trn_rl_repo"):
    if os.path.isdir(_p) and _p not in sys.path:
        sys.path.insert(0, _p)

import concourse.bass as bass
import concourse.bacc as bacc
import concourse.mybir as mybir
import concourse.tile as tile
from concourse.masks import make_identity

B, S, HID = 4, 2048, 1024
NH, DH = 16, 64
JC = 512                 # features per core (8 heads)
NCORES = 8
FP = mybir.dt.float32
BF = mybir.dt.bfloat16
I32 = mybir.dt.int32
MULT = mybir.AluOpType.mult
ADD = mybir.AluOpType.add

TRACE = False            # set True (e.g. from test.py) to neuron-profile
LAST_RESULTS = {}        # exec_time_ns etc. stashed here for the harness


def build_nc():
    nc = bacc.Bacc()
    x = nc.declare_dram_parameter("x", [S, HID], FP, isOutput=False)
    mask = nc.declare_dram_parameter("mask", [S], I32, isOutput=False)
    wq = nc.declare_dram_parameter("wq", [JC, HID], FP, isOutput=False)
    bq = nc.declare_dram_parameter("bq", [JC], FP, isOutput=False)
    wk = nc.declare_dram_parameter("wk", [JC, HID], FP, isOutput=False)
    bk = nc.declare_dram_parameter("bk", [JC], FP, isOutput=False)
    wv = nc.declare_dram_parameter("wv", [JC, HID], FP, isOutput=False)
    bv = nc.declare_dram_parameter("bv", [JC], FP, isOutput=False)
    wo = nc.declare_dram_parameter("wo", [HID, JC], FP, isOutput=False)
    bo = nc.declare_dram_parameter("bo", [HID], FP, isOutput=False)
    y = nc.declare_dram_parameter("y", [HID, S], FP, isOutput=True)

    xr = x.rearrange("t i -> i t")        # [HID, S] feature-major view

    with tile.TileContext(nc) as tc, ExitStack() as ctx:
        const = ctx.enter_context(tc.tile_pool(name="const", bufs=1))
        mid = ctx.enter_context(tc.tile_pool(name="mid", bufs=1))

        ident = const.tile([128, 128], FP)
        make_identity(nc, ident)
        ones1 = const.tile([1, 64], FP)
        nc.vector.memset(ones1[:], 1.0)

        # mask -> additive bias maskA[p, kt] = 0 (keep) / -1e9 (drop)
        mask_i = const.tile([128, 16], I32)
        nc.sync.dma_start(out=mask_i[:], in_=mask.rearrange("(kt p) -> p kt", p=128))
        mask_f = const.tile([128, 16], FP)
        nc.vector.tensor_copy(out=mask_f[:], in_=mask_i[:])
        maskA = const.tile([128, 16], FP)
        nc.vector.tensor_scalar(maskA[:], mask_f[:], 1e9, -1e9, MULT, ADD)

        # biases [JC] -> [128, 4]
        def load_bias_jc(b_dram):
            t = const.tile([128, 4], FP, tag=f"b_{b_dram.name}")
            nc.sync.dma_start(out=t[:], in_=b_dram.rearrange("(o p) -> p o", p=128))
            return t

        bqt, bkt, bvt = load_bias_jc(bq), load_bias_jc(bk), load_bias_jc(bv)
        bot = const.tile([128, 8], FP)
        nc.sync.dma_start(out=bot[:], in_=bo.rearrange("(o p) -> p o", p=128))

        # persistent mid tensors
        KT = mid.tile([128, 4, S], FP)           # K^T: [j within tile, jt, t]
        vaug = mid.tile([128, 16, 8, 65], BF)    # V augmented: [k, kt, head, d|1]
        nc.vector.memset(vaug[:, :, :, 64:65], 1.0)
        outT = mid.tile([128, 4, S], FP)         # attention out^T (c-major)

        # ---------------- Phase A: K^T and V(aug) for all tokens ----------------
        with ExitStack() as actx:
            wpool = actx.enter_context(tc.tile_pool(name="wpool", bufs=1))
            xpool = actx.enter_context(tc.tile_pool(name="xpool", bufs=2))
            vtpool = actx.enter_context(tc.tile_pool(name="vtpool", bufs=2))
            ppsum = actx.enter_context(tc.tile_pool(name="ppsum", bufs=2, space="PSUM"))
            tpsum = actx.enter_context(tc.tile_pool(name="tpsum", bufs=2, space="PSUM"))

            wkT = wpool.tile([128, 8, JC], FP, tag="wkT")
            wvT = wpool.tile([128, 8, JC], FP, tag="wvT")
            for wt, wd in ((wkT, wk), (wvT, wv)):
                wr = wd.rearrange("j i -> i j")
                for it in range(8):
                    nc.sync.dma_start(out=wt[:, it, :], in_=wr[it * 128:(it + 1) * 128, :])

            for tq in range(4):                   # token quarters of 512
                t0 = tq * 512
                xT = xpool.tile([128, 8, 512], FP, tag="xT")
                for it in range(8):
                    nc.sync.dma_start(
                        out=xT[:, it, :], in_=xr[it * 128:(it + 1) * 128, t0:t0 + 512]
                    )
                for jt in range(4):
                    # K projection
                    kps = ppsum.tile([128, 512], FP, tag="kps")
                    for it in range(8):
                        nc.tensor.matmul(
                            kps[:], lhsT=wkT[:, it, jt * 128:(jt + 1) * 128],
                            rhs=xT[:, it, :], start=(it == 0), stop=(it == 7),
                        )
                    nc.vector.tensor_scalar_add(
                        KT[:, jt, t0:t0 + 512], kps[:], bkt[:, jt:jt + 1]
                    )
                    # V projection -> bias add -> transpose into vaug
                    vps = ppsum.tile([128, 512], FP, tag="vps")
                    for it in range(8):
                        nc.tensor.matmul(
                            vps[:], lhsT=wvT[:, it, jt * 128:(jt + 1) * 128],
                            rhs=xT[:, it, :], start=(it == 0), stop=(it == 7),
                        )
                    vtmp = vtpool.tile([128, 512], FP, tag="vtmp")
                    nc.vector.tensor_scalar_add(vtmp[:], vps[:], bvt[:, jt:jt + 1])
                    for hh in range(2):           # head-halves within the j-tile
                        head = jt * 2 + hh
                        for ktt in range(4):      # k-tiles within the quarter
                            kt = tq * 4 + ktt
                            tp = tpsum.tile([128, 64], FP, tag="tp")
                            nc.tensor.transpose(
                                tp[:, :],
                                vtmp[hh * 64:(hh + 1) * 64, ktt * 128:(ktt + 1) * 128],
                                ident[hh * 64:(hh + 1) * 64, hh * 64:(hh + 1) * 64],
                            )
                            nc.vector.tensor_copy(
                                out=vaug[:, kt, head, 0:64], in_=tp[:, :]
                            )

        # ---------------- Phase B: per q-half Q^T + attention ----------------
        with ExitStack() as bctx:
            wqpool = bctx.enter_context(tc.tile_pool(name="wqpool", bufs=1))
            qtpool = bctx.enter_context(tc.tile_pool(name="qtpool", bufs=2))

            wqT = wqpool.tile([128, 8, JC], FP, tag="wqT")
            wr = wq.rearrange("j i -> i j")
            for it in range(8):
                nc.sync.dma_start(out=wqT[:, it, :], in_=wr[it * 128:(it + 1) * 128, :])

            for qh in range(2):                  # q halves of 1024
                q0 = qh * 1024
                QTh = qtpool.tile([128, 4, 1024], FP, tag="QTh")
                with ExitStack() as qctx:
                    xqpool = qctx.enter_context(tc.tile_pool(name="xqpool", bufs=2))
                    qpsum = qctx.enter_context(
                        tc.tile_pool(name="qpsum", bufs=2, space="PSUM")
                    )
                    for tq in range(2):          # quarters of the half
                        xTq = xqpool.tile([128, 8, 512], FP, tag="xTq")
                        for it in range(8):
                            nc.sync.dma_start(
                                out=xTq[:, it, :],
                                in_=xr[it * 128:(it + 1) * 128,
                                       q0 + tq * 512:q0 + (tq + 1) * 512],
                            )
                        for jt in range(4):
                            qps = qpsum.tile([128, 512], FP, tag="qps")
                            for it in range(8):
                                nc.tensor.matmul(
                                    qps[:], lhsT=wqT[:, it, jt * 128:(jt + 1) * 128],
                                    rhs=xTq[:, it, :], start=(it == 0), stop=(it == 7),
                                )
                            nc.vector.tensor_scalar_add(
                                QTh[:, jt, tq * 512:(tq + 1) * 512],
                                qps[:], bqt[:, jt:jt + 1],
                            )

                with ExitStack() as attx:
                    ptpool = attx.enter_context(tc.tile_pool(name="ptpool", bufs=2))
                    rpool = attx.enter_context(tc.tile_pool(name="rpool", bufs=3))
                    spool = attx.enter_context(
                        tc.tile_pool(name="spool", bufs=2, space="PSUM")
                    )
                    avpool = attx.enter_context(
                        tc.tile_pool(name="avpool", bufs=2, space="PSUM")
                    )
                    for h in range(8):
                        jt, hh = h // 2, h % 2
                        base = hh * 64
                        avp = avpool.tile([128, 1024], FP, tag="av")
                        for half_k in range(2):
                            PTt = ptpool.tile([128, 8, 1024], BF, tag="PT")
                            for kk in range(8):
                                kt = half_k * 8 + kk
                                sps = spool.tile([128, 1024], FP, tag="sp")
                                for qq in range(2):
                                    nc.tensor.matmul(
                                        sps[:, qq * 512:(qq + 1) * 512],
                                        lhsT=KT[base:base + 64, jt,
                                                kt * 128:(kt + 1) * 128],
                                        rhs=QTh[base:base + 64, jt,
                                                qq * 512:(qq + 1) * 512],
                                        start=True, stop=True,
                                    )
                                nc.scalar.activation(
                                    PTt[:, kk, :], sps[:],
                                    mybir.ActivationFunctionType.Exp,
                                    bias=maskA[:, kt:kt + 1], scale=0.125,
                                )
                            for kk in range(8):
                                kt = half_k * 8 + kk
                                for qq in range(2):
                                    nc.tensor.matmul(
                                        avp[0:65, qq * 512:(qq + 1) * 512],
                                        lhsT=vaug[:, kt, h, :],
                                        rhs=PTt[:, kk, qq * 512:(qq + 1) * 512],
                                        start=(kt == 0), stop=(kt == 15),
                                        skip_group_check=True,
                                    )
                        # normalize: sums row 64 -> reciprocal broadcast -> multiply
                        s_sb = rpool.tile([1, 1024], FP, tag="s_sb")
                        nc.vector.tensor_copy(out=s_sb[:], in_=avp[64:65, :])
                        sums_b = spool.tile([128, 1024], FP, tag="sp")
                        for qq in range(2):
                            nc.tensor.matmul(
                                sums_b[0:64, qq * 512:(qq + 1) * 512],
                                lhsT=ones1[:], rhs=s_sb[:, qq * 512:(qq + 1) * 512],
                                start=True, stop=True,
                            )
                        recb = rpool.tile([64, 1024], FP, tag="recb")
                        nc.vector.reciprocal(recb[:], sums_b[0:64, :])
                        nc.vector.tensor_tensor(
                            outT[base:base + 64, jt, q0:q0 + 1024],
                            avp[0:64, :], recb[:], MULT,
                        )

        # ---------------- Phase C: partial out-projection ----------------
        with ExitStack() as cctx:
            wopool = cctx.enter_context(tc.tile_pool(name="wopool", bufs=1))
            ypool = cctx.enter_context(tc.tile_pool(name="ypool", bufs=4))
            ypsum = cctx.enter_context(tc.tile_pool(name="ypsum", bufs=4, space="PSUM"))

            woT = wopool.tile([128, 4, HID], FP, tag="woT")
            wor = wo.rearrange("o c -> c o")
            for ct in range(4):
                nc.sync.dma_start(out=woT[:, ct, :], in_=wor[ct * 128:(ct + 1) * 128, :])

            for ot in range(8):
                for tc_i in range(4):
                    yps = ypsum.tile([128, 512], FP, tag="yps")
                    for ct in range(4):
                        nc.tensor.matmul(
                            yps[:], lhsT=woT[:, ct, ot * 128:(ot + 1) * 128],
                            rhs=outT[:, ct, tc_i * 512:(tc_i + 1) * 512],
                            start=(ct == 0), stop=(ct == 3),
                        )
                    yt = ypool.tile([128, 512], FP, tag="yt")
                    nc.vector.tensor_scalar_add(yt[:], yps[:], bot[:, ot:ot + 1])
                    nc.sync.dma_start(
                        out=y[ot * 128:(ot + 1) * 128, tc_i * 512:(tc_i + 1) * 512],
                        in_=yt[:],
                    )
    return nc


_NC = None


def _get_nc():
    global _NC
    if _NC is None:
        _NC = build_nc()
        _NC.finalize()   # run Bacc passes (reg alloc, wait splitting)
    return _NC


def make_in_maps(x, mask, Wq, bq, Wk, bk, Wv, bv, Wo, bo):
    f32 = lambda a: np.ascontiguousarray(np.asarray(a, dtype=np.float32))
    in_maps = []
    for c in range(NCORES):
        b, g = c // 2, c % 2
        sl = slice(g * JC, (g + 1) * JC)
        in_maps.append({
            "x": f32(x[b]),
            "mask": np.ascontiguousarray(np.asarray(mask[b], dtype=np.int32)),
            "wq": f32(Wq[sl]), "bq": f32(bq[sl]),
            "wk": f32(Wk[sl]), "bk": f32(bk[sl]),
            "wv": f32(Wv[sl]), "bv": f32(bv[sl]),
            "wo": f32(Wo[:, sl]),
            "bo": f32(bo) if g == 0 else np.zeros(HID, np.float32),
        })
    return in_maps


def kernel(x, mask, Wq, bq, Wk, bk, Wv, bv, Wo, bo):
    from concourse.bass_utils import run_bass_kernel_spmd

    nc = _get_nc()
    in_maps = make_in_maps(x, mask, Wq, bq, Wk, bk, Wv, bv, Wo, bo)
    kw = {}
    if TRACE:
        import os as _os
        _os.makedirs("/root/problem/trace_out", exist_ok=True)
        kw = dict(tmpdir="/root/problem/trace_out")
    r = run_bass_kernel_spmd(nc, in_maps, list(range(NCORES)), trace=TRACE, **kw)
    LAST_RESULTS["exec_time_ns"] = r.exec_time_ns
    LAST_RESULTS["mean_exec_time_ns"] = r.mean_exec_time_ns
    y = np.empty((B, S, HID), np.float32)
    for b in range(B):
        y[b] = (r.results[2 * b]["y"] + r.results[2 * b + 1]["y"]).T
    return y


# revision 10
# speedup vs baseline: 4.0713x; 4.0713x over previous
"""Trainium2 Bass kernel for nn_AttentionModeEncoder (B=4, S=2048, HID=1024, 16 heads x 64).

Sharding: 8 cores = 4 batches x 2 head-groups (8 heads / 512 features per core).
Per core (batch b, head-group g):
  Phase A: x loaded CONTIGUOUSLY row-major, transposed on PE to x^T; Q^T/K^T/V
    projections (fp32) with weights also PE-transposed from contiguous loads.
    V goes into a ones-augmented bf16 [k, head, d|1] layout for the AV matmul.
  Phase B: attention per (head, 1024-wide q chunk) in transposed layout:
    S^T[k,q] = K^T.T @ Q^T (d=64 contraction), P = exp(0.125*S + maskbias) on
    ScalarE with the additive mask as per-partition bias (bf16 out), AV with the
    ones row giving softmax denominators for free, PE outer-product broadcast +
    fast reciprocal + DVE multiply for the normalize (bf16 out^T).
  Phase C: partial out-projection y^T = Wo[:, cslice] @ attn^T (bf16 matmul,
    fp32 accumulate + bias) streamed to DRAM.
Host sums the two partials per batch (the cross-head-group reduction).
"""

import os
import sys
import numpy as np
from contextlib import ExitStack

for _p in ("/opt/trn_rl_repo", "/root/.axon_site/_ro/trn_rl_repo"):
    if os.path.isdir(_p) and _p not in sys.path:
        sys.path.insert(0, _p)

import concourse.bass as bass
import concourse.bacc as bacc
import concourse.mybir as mybir
import concourse.tile as tile
from concourse.masks import make_identity

B, S, HID = 4, 2048, 1024
JC = 512                 # features per core (8 heads)
NCORES = 8
FP = mybir.dt.float32
BF = mybir.dt.bfloat16
I32 = mybir.dt.int32
MULT = mybir.AluOpType.mult
ADD = mybir.AluOpType.add

TRACE = False
LAST_RESULTS = {}


def build_nc():
    nc = bacc.Bacc()
    x = nc.declare_dram_parameter("x", [S, HID], FP, isOutput=False)
    mask = nc.declare_dram_parameter("mask", [S], I32, isOutput=False)
    wq = nc.declare_dram_parameter("wq", [JC, HID], FP, isOutput=False)
    bq = nc.declare_dram_parameter("bq", [JC], FP, isOutput=False)
    wk = nc.declare_dram_parameter("wk", [JC, HID], FP, isOutput=False)
    bk = nc.declare_dram_parameter("bk", [JC], FP, isOutput=False)
    wv = nc.declare_dram_parameter("wv", [JC, HID], FP, isOutput=False)
    bv = nc.declare_dram_parameter("bv", [JC], FP, isOutput=False)
    wo = nc.declare_dram_parameter("wo", [HID, JC], FP, isOutput=False)
    bo = nc.declare_dram_parameter("bo", [HID], FP, isOutput=False)
    y = nc.declare_dram_parameter("y", [HID, S], FP, isOutput=True)

    with tile.TileContext(nc) as tc, ExitStack() as ctx:
        const = ctx.enter_context(tc.tile_pool(name="const", bufs=1))
        mid = ctx.enter_context(tc.tile_pool(name="mid", bufs=1))

        ident = const.tile([128, 128], FP)
        make_identity(nc, ident)
        ones1 = const.tile([1, 64], FP)
        nc.vector.memset(ones1[:], 1.0)

        # mask -> additive bias maskA[p, kt] = 0 (keep) / -1e9 (drop)
        mask_i = const.tile([128, 16], I32)
        nc.sync.dma_start(out=mask_i[:], in_=mask.rearrange("(kt p) -> p kt", p=128))
        mask_f = const.tile([128, 16], FP)
        nc.vector.tensor_copy(out=mask_f[:], in_=mask_i[:])
        maskA = const.tile([128, 16], FP)
        nc.vector.tensor_scalar(maskA[:], mask_f[:], 1e9, -1e9, MULT, ADD)

        def load_bias_jc(b_dram):
            t = const.tile([128, 4], FP, tag=f"b_{b_dram.name}")
            nc.sync.dma_start(out=t[:], in_=b_dram.rearrange("(o p) -> p o", p=128))
            return t

        bqt, bkt, bvt = load_bias_jc(bq), load_bias_jc(bk), load_bias_jc(bv)
        bot = const.tile([128, 8], FP)
        nc.sync.dma_start(out=bot[:], in_=bo.rearrange("(o p) -> p o", p=128))

        # persistent tensors
        KT = mid.tile([128, 4, S], FP)           # K^T: [j in tile, jt, t]
        QT = mid.tile([128, 4, S], FP)           # Q^T
        vaug = mid.tile([128, 16, 8, 65], BF)    # V aug: [k, kt, head, d|1]
        nc.vector.memset(vaug[:, :, :, 64:65], 1.0)
        outT = mid.tile([128, 4, S], BF)         # attention out^T (c-major)

        # ------------- Phase A: x^T then Q^T/K^T/V projections -------------
        with ExitStack() as actx:
            xtp = actx.enter_context(tc.tile_pool(name="xtp", bufs=1))
            xT = xtp.tile([128, 8, S], FP)       # [i in tile, it, t] 64KB/part

            with ExitStack() as a1ctx:
                xrowp = a1ctx.enter_context(tc.tile_pool(name="xrowp", bufs=2))
                tpsA = a1ctx.enter_context(
                    tc.tile_pool(name="tpsA", bufs=3, space="PSUM")
                )
                for tq in range(4):
                    t0 = tq * 512
                    xrow = xrowp.tile([128, 4, HID], FP, tag="xrow")
                    nc.sync.dma_start(
                        out=xrow[:],
                        in_=x[t0:t0 + 512, :].rearrange("(a p) i -> p a i", p=128),
                    )
                    for it in range(8):
                        for a in range(4):
                            tp = tpsA.tile([128, 128], FP, tag="tp")
                            nc.tensor.transpose(
                                tp[:], xrow[:, a, it * 128:(it + 1) * 128], ident[:]
                            )
                            nc.vector.tensor_copy(
                                out=xT[:, it, t0 + a * 128:t0 + (a + 1) * 128],
                                in_=tp[:],
                            )

            with ExitStack() as a2ctx:
                wrp = a2ctx.enter_context(tc.tile_pool(name="wrp", bufs=1))
                wtp = a2ctx.enter_context(tc.tile_pool(name="wtp", bufs=1))
                vtp = a2ctx.enter_context(tc.tile_pool(name="vtp", bufs=2))
                pps = a2ctx.enter_context(
                    tc.tile_pool(name="pps", bufs=1, space="PSUM")
                )
                tpsW = a2ctx.enter_context(
                    tc.tile_pool(name="tpsW", bufs=3, space="PSUM")
                )

                for wd, bt, kind in ((wk, bkt, "K"), (wv, bvt, "V"), (wq, bqt, "Q")):
                    wrow = wrp.tile([128, 4, HID], FP, tag="wrow")
                    nc.sync.dma_start(
                        out=wrow[:], in_=wd.rearrange("(a p) i -> p a i", p=128)
                    )
                    wT = wtp.tile([128, 8, JC], FP, tag="wT")
                    for it in range(8):
                        for a in range(4):
                            tp = tpsW.tile([128, 128], FP, tag="tpw")
                            nc.tensor.transpose(
                                tp[:], wrow[:, a, it * 128:(it + 1) * 128], ident[:]
                            )
                            nc.vector.tensor_copy(
                                out=wT[:, it, a * 128:(a + 1) * 128], in_=tp[:]
                            )
                    for jt in range(4):
                        psums = [
                            pps.tile([128, 512], FP, tag=f"pp{i}", name=f"pp{i}")
                            for i in range(4)
                        ]
                        for it in range(8):
                            for tq in range(4):
                                nc.tensor.matmul(
                                    psums[tq][:],
                                    lhsT=wT[:, it, jt * 128:(jt + 1) * 128],
                                    rhs=xT[:, it, tq * 512:(tq + 1) * 512],
                                    start=(it == 0), stop=(it == 7),
                                )
                        for tq in range(4):
                            t0 = tq * 512
                            if kind == "K":
                                nc.vector.tensor_scalar_add(
                                    KT[:, jt, t0:t0 + 512], psums[tq][:],
                                    bt[:, jt:jt + 1],
                                )
                            elif kind == "Q":
                                nc.vector.tensor_scalar_add(
                                    QT[:, jt, t0:t0 + 512], psums[tq][:],
                                    bt[:, jt:jt + 1],
                                )
                            else:
                                vtmp = vtp.tile([128, 512], FP, tag="vtmp")
                                nc.vector.tensor_scalar_add(
                                    vtmp[:], psums[tq][:], bt[:, jt:jt + 1]
                                )
                                for hh in range(2):
                                    head = jt * 2 + hh
                                    for ktt in range(4):
                                        kt = tq * 4 + ktt
                                        tp = tpsW.tile([128, 64], FP, tag="tpw")
                                        nc.tensor.transpose(
                                            tp[0:128, 0:64],
                                            vtmp[hh * 64:(hh + 1) * 64,
                                                 ktt * 128:(ktt + 1) * 128],
                                            ident[hh * 64:(hh + 1) * 64,
                                                  hh * 64:(hh + 1) * 64],
                                        )
                                        nc.vector.tensor_copy(
                                            out=vaug[:, kt, head, 0:64],
                                            in_=tp[0:128, 0:64],
                                        )

        # ------------- Phase B: attention -------------
        with ExitStack() as bctx:
            ptpool = bctx.enter_context(tc.tile_pool(name="ptpool", bufs=3))
            rpool = bctx.enter_context(tc.tile_pool(name="rpool", bufs=3))
            spool = bctx.enter_context(tc.tile_pool(name="spool", bufs=2, space="PSUM"))
            avpool = bctx.enter_context(
                tc.tile_pool(name="avpool", bufs=2, space="PSUM")
            )
            for h in range(8):
                jt, hh = h // 2, h % 2
                base = hh * 64
                for qc in range(2):              # q chunks of 1024
                    q0 = qc * 1024
                    avp = avpool.tile([128, 1024], FP, tag="av")
                    for half_k in range(2):
                        PTt = ptpool.tile([128, 8, 1024], BF, tag="PT")
                        for kk in range(8):
                            kt = half_k * 8 + kk
                            sps = spool.tile([128, 1024], FP, tag="sp")
                            for qq in range(2):
                                nc.tensor.matmul(
                                    sps[:, qq * 512:(qq + 1) * 512],
                                    lhsT=KT[base:base + 64, jt,
                                            kt * 128:(kt + 1) * 128],
                                    rhs=QT[base:base + 64, jt,
                                           q0 + qq * 512:q0 + (qq + 1) * 512],
                                    start=True, stop=True,
                                )
                            nc.scalar.activation(
                                PTt[:, kk, :], sps[:],
                                mybir.ActivationFunctionType.Exp,
                                bias=maskA[:, kt:kt + 1], scale=0.125,
                            )
                        for kk in range(8):
                            kt = half_k * 8 + kk
                            for qq in range(2):
                                nc.tensor.matmul(
                                    avp[0:65, qq * 512:(qq + 1) * 512],
                                    lhsT=vaug[:, kt, h, :],
                                    rhs=PTt[:, kk, qq * 512:(qq + 1) * 512],
                                    start=(kt == 0), stop=(kt == 15),
                                    skip_group_check=True,
                                )
                    # normalize
                    s_sb = rpool.tile([1, 1024], FP, tag="s_sb")
                    nc.vector.tensor_copy(out=s_sb[:], in_=avp[64:65, :])
                    sums_b = spool.tile([128, 1024], FP, tag="sp")
                    for qq in range(2):
                        nc.tensor.matmul(
                            sums_b[0:64, qq * 512:(qq + 1) * 512],
                            lhsT=ones1[:], rhs=s_sb[:, qq * 512:(qq + 1) * 512],
                            start=True, stop=True,
                        )
                    recb = rpool.tile([64, 1024], FP, tag="recb")
                    nc.vector.reciprocal_approx_fast(recb[:], sums_b[0:64, :])
                    nc.vector.tensor_tensor(
                        outT[base:base + 64, jt, q0:q0 + 1024],
                        avp[0:64, :], recb[:], MULT,
                    )

        # ------------- Phase C: partial out-projection -------------
        with ExitStack() as cctx:
            worp = cctx.enter_context(tc.tile_pool(name="worp", bufs=1))
            wotp = cctx.enter_context(tc.tile_pool(name="wotp", bufs=1))
            ypool = cctx.enter_context(tc.tile_pool(name="ypool", bufs=4))
            ypsum = cctx.enter_context(tc.tile_pool(name="ypsum", bufs=2, space="PSUM"))
            tpsC = cctx.enter_context(tc.tile_pool(name="tpsC", bufs=3, space="PSUM"))

            worow = worp.tile([128, 8, JC], FP)      # [o-part, a, c]
            nc.sync.dma_start(
                out=worow[:], in_=wo.rearrange("(a p) c -> p a c", p=128)
            )
            woT = wotp.tile([128, 4, HID], BF)       # [c-part, ct, o]
            for ct in range(4):
                for a in range(8):
                    tp = tpsC.tile([128, 128], FP, tag="tpc")
                    nc.tensor.transpose(
                        tp[:], worow[:, a, ct * 128:(ct + 1) * 128], ident[:]
                    )
                    nc.vector.tensor_copy(
                        out=woT[:, ct, a * 128:(a + 1) * 128], in_=tp[:]
                    )

            for ot in range(8):
                for tc_i in range(2):                # t chunks of 1024
                    yps = ypsum.tile([128, 1024], FP, tag="yps")
                    for ct in range(4):
                        for qq in range(2):
                            nc.tensor.matmul(
                                yps[:, qq * 512:(qq + 1) * 512],
                                lhsT=woT[:, ct, ot * 128:(ot + 1) * 128],
                                rhs=outT[:, ct,
                                         tc_i * 1024 + qq * 512:
                                         tc_i * 1024 + (qq + 1) * 512],
                                start=(ct == 0), stop=(ct == 3),
                            )
                    yt = ypool.tile([128, 1024], FP, tag="yt")
                    nc.vector.tensor_scalar_add(yt[:], yps[:], bot[:, ot:ot + 1])
                    nc.sync.dma_start(
                        out=y[ot * 128:(ot + 1) * 128,
                              tc_i * 1024:(tc_i + 1) * 1024],
                        in_=yt[:],
                    )
    return nc


_NC = None


def _get_nc():
    global _NC
    if _NC is None:
        _NC = build_nc()
        _NC.finalize()   # run Bacc passes (reg alloc, wait splitting)
    return _NC


def make_in_maps(x, mask, Wq, bq, Wk, bk, Wv, bv, Wo, bo):
    f32 = lambda a: np.ascontiguousarray(np.asarray(a, dtype=np.float32))
    in_maps = []
    for c in range(NCORES):
        b, g = c // 2, c % 2
        sl = slice(g * JC, (g + 1) * JC)
        in_maps.append({
            "x": f32(x[b]),
            "mask": np.ascontiguousarray(np.asarray(mask[b], dtype=np.int32)),
            "wq": f32(Wq[sl]), "bq": f32(bq[sl]),
            "wk": f32(Wk[sl]), "bk": f32(bk[sl]),
            "wv": f32(Wv[sl]), "bv": f32(bv[sl]),
            "wo": f32(Wo[:, sl]),
            "bo": f32(bo) if g == 0 else np.zeros(HID, np.float32),
        })
    return in_maps


def kernel(x, mask, Wq, bq, Wk, bk, Wv, bv, Wo, bo):
    from concourse.bass_utils import run_bass_kernel_spmd

    nc = _get_nc()
    in_maps = make_in_maps(x, mask, Wq, bq, Wk, bk, Wv, bv, Wo, bo)
    kw = {}
    if TRACE:
        os.makedirs("/root/problem/trace_out", exist_ok=True)
        kw = dict(tmpdir="/root/problem/trace_out")
    r = run_bass_kernel_spmd(nc, in_maps, list(range(NCORES)), trace=TRACE, **kw)
    LAST_RESULTS["exec_time_ns"] = r.exec_time_ns
    LAST_RESULTS["mean_exec_time_ns"] = r.mean_exec_time_ns
    y = np.empty((B, S, HID), np.float32)
    for b in range(B):
        y[b] = (r.results[2 * b]["y"] + r.results[2 * b + 1]["y"]).T
    return y
